# revision 1
# baseline (speedup 1.0000x reference)
"""Compact Bilinear Pooling (count-sketch + circular conv + spatial sum-pool)
as a Trainium2 Bass/Tile kernel, SPMD over 8 NeuronCores.

Math: with sk_i = flat @ S_i (flat: [B*P, C]), the reference computes
    out[b] = sum_{p in sample b} ifft( fft(sk1_p) * fft(sk2_p) ).real
Fold the (constant) sketch matrices into the DFT:  Phi_i = fft(S_i, axis=1),
so fft(sk_i,p) = x_p^T Phi_i.  Because sk are real, only the half spectrum
k = 0..D/2 is needed:
    Shat[b,k]  = sum_p (x_p^T Phi1[:,k]) * (x_p^T Phi2[:,k])
    out[b,d]   = sum_k wk*( Re Shat * cos(2 pi k d/D) - Im Shat * sin(...) )/D
Device pipeline per core (sharded by frequency: 512 of 4096 padded freqs):
  1. Ghat tiles [128 pos, 512 freq] via f32r matmuls (full PE rate).
  2. complex products on DVE; per-sample position-reduction via +-1
     indicator matmuls accumulating into one PSUM bank -> Shat [16, 512].
  3. inverse DFT as bf16 matmul against precomputed cos/sin slabs,
     partial [16, 8000] per core; host sums the 8 partials.
"""

import numpy as np
import ml_dtypes

import concourse.bacc as bacc
import concourse.mybir as mybir
import concourse.tile as tile
from concourse.bass_utils import run_bass_kernel_spmd

# problem dims (hardcoded per spec)
B, C, H, W, D = 16, 512, 14, 14, 8000
P = H * W            # 196 positions per sample
BP = B * P           # 3136
KH = D // 2 + 1      # 4001 half-spectrum frequencies
KPAD = 4096          # padded to 8*512
NCORES = 8
KSL = KPAD // NCORES  # 512 freqs per core
NCC = C // 128        # 4 contraction chunks
NPT = (BP + 127) // 128  # 25 position tiles (24x128 + 64)
DQ = D // 4           # 2000 output cols per quarter
DCH = 500             # inverse matmul free dim (4 chunks per quarter)
NKT = KSL // 128      # 4 k-chunks of the core's freq slice

F32R = mybir.dt.float32r
F32 = mybir.dt.float32
BF16 = mybir.dt.bfloat16


def build_nc():
    nc = bacc.Bacc("TRN2", target_bir_lowering=False, debug=False)
    x_d = nc.dram_tensor("x", [B, C, P], F32R, kind="ExternalInput")
    phi_d = nc.dram_tensor("phi", [128, NCC * 4 * KSL], F32R, kind="ExternalInput")
    ind_d = nc.dram_tensor("ind", [128, 3 * NPT * 2 * B], F32R, kind="ExternalInput")
    cc_d = nc.dram_tensor("cc", [2, NKT, 128, D], BF16, kind="ExternalInput")
    out_d = nc.dram_tensor("out", [B, D], F32, kind="ExternalOutput")

    xa = x_d.ap().rearrange("b c p -> c b p")  # [512, 16, 196]

    with tile.TileContext(nc) as tc:
        with (
            tc.tile_pool(name="phi", bufs=1) as phi_pool,
            tc.tile_pool(name="xin", bufs=1) as x_pool,
            tc.tile_pool(name="bcp", bufs=4) as b_pool,
            tc.tile_pool(name="prd", bufs=8) as prod_pool,
            tc.tile_pool(name="sbf", bufs=1) as s_pool,
            tc.tile_pool(name="cslab", bufs=16) as c_pool,
            tc.tile_pool(name="stage", bufs=2) as st_pool,
            tc.tile_pool(name="mm", bufs=5, space="PSUM") as mm_psum,
            tc.tile_pool(name="sac", bufs=1, space="PSUM") as s_psum,
            tc.tile_pool(name="inv", bufs=2, space="PSUM") as inv_psum,
        ):
            # ---- constants: phi (4 chunks), indicators, x (4 c-chunks)
            phit = phi_pool.tile([128, NCC * 4 * KSL], F32R)
            for i in range(4):
                sl = slice(i * 2048, (i + 1) * 2048)
                nc.sync.dma_start(phit[:, sl], phi_d.ap()[:, sl])
            indt = phi_pool.tile([128, 3 * NPT * 2 * B], F32R, tag="ind")
            nc.sync.dma_start(indt[:], ind_d.ap())

            xt = []
            for cci in range(NCC):
                t = x_pool.tile([128, B, P], F32R, tag=f"x{cci}")
                nc.sync.dma_start(t[:], xa[cci * 128:(cci + 1) * 128])
                xt.append(t[:].rearrange("c b p -> c (b p)"))

            # ---- Shat accumulator: rows 0:16 = Re, rows 16:32 = Im
            s_acc = s_psum.tile([32, KSL], F32, tag="sacc")

            # ---- main stage
            for pt in range(NPT):
                M = min(128, BP - pt * 128)
                ps = []
                for m in range(4):  # 0:g1re 1:g1im 2:g2re 3:g2im
                    g = mm_psum.tile([128, KSL], F32, tag="mm")
                    for cci in range(NCC):
                        nc.tensor.matmul(
                            g[0:M, :],
                            lhsT=xt[cci][:, pt * 128:pt * 128 + M],
                            rhs=phit[:, (cci * 4 + m) * KSL:(cci * 4 + m + 1) * KSL],
                            start=(cci == 0),
                            stop=(cci == NCC - 1),
                        )
                    ps.append(g)
                b2re = b_pool.tile([128, KSL], F32, tag="b2re")
                nc.scalar.copy(b2re[0:M, :], ps[2][0:M, :])
                b2im = b_pool.tile([128, KSL], F32, tag="b2im")
                nc.scalar.copy(b2im[0:M, :], ps[3][0:M, :])

                prods = []
                for in0, in1 in ((ps[0], b2re), (ps[1], b2im),
                                 (ps[0], b2im), (ps[1], b2re)):
                    pr = prod_pool.tile([128, KSL], F32R, tag="prod")
                    nc.vector.tensor_mul(pr[0:M, :], in0[0:M, :], in1[0:M, :])
                    prods.append(pr)

                # per-sample reduce over positions: S += ind^T @ prod
                # combined [M, 32] indicators: Re rows 0:16, Im rows 16:32
                # patterns: 0 = [+1|0] (RR), 1 = [-1|0] (II), 2 = [0|+1] (RI, IR)
                for i, (pr, pat) in enumerate((
                    (prods[0], 0), (prods[1], 1), (prods[2], 2), (prods[3], 2),
                )):
                    off = (pat * NPT + pt) * 2 * B
                    nc.tensor.matmul(
                        s_acc[:],
                        lhsT=indt[0:M, off:off + 2 * B],
                        rhs=pr[0:M, :],
                        start=(pt == 0 and i == 0),
                        stop=(pt == NPT - 1 and i == 3),
                        skip_group_check=True,
                    )

            # ---- Shat -> bf16, transpose to [freq, sample] via DMA transpose
            s_bf = s_pool.tile([32, KSL], BF16, tag="sbf")
            nc.scalar.copy(s_bf[:], s_acc[:])
            sT = []
            for kt in range(NKT):
                t = s_pool.tile([128, 32], BF16, tag=f"sT{kt}")
                nc.sync.dma_start(t[:], s_bf[:, kt * 128:(kt + 1) * 128],
                                  transpose=True)
                sT.append(t)

            # ---- inverse: out[b,d] = sum_k Sre*Cre + Sim*Cim  (bf16 matmuls)
            for q in range(4):
                slabs = {}
                for t in range(2):
                    for kt in range(NKT):
                        st = c_pool.tile([128, DQ], BF16, tag="cslab")
                        nc.sync.dma_start(
                            st[:], cc_d.ap()[t, kt, :, q * DQ:(q + 1) * DQ]
                        )
                        slabs[(t, kt)] = st
                stage = st_pool.tile([B, DQ], F32, tag="stage")
                for dq in range(4):
                    pinv = inv_psum.tile([B, DCH], F32, tag="inv")
                    idx = 0
                    for t, col in ((0, 0), (1, 16)):
                        for kt in range(NKT):
                            nc.tensor.matmul(
                                pinv[:],
                                lhsT=sT[kt][:, col:col + B],
                                rhs=slabs[(t, kt)][:, dq * DCH:(dq + 1) * DCH],
                                start=(idx == 0),
                                stop=(idx == 7),
                            )
                            idx += 1
                    nc.scalar.copy(stage[:, dq * DCH:(dq + 1) * DCH], pinv[:])
                nc.sync.dma_start(out_d.ap()[:, q * DQ:(q + 1) * DQ], stage[:])

    nc.compile()
    return nc


def make_constants(S1, S2):
    """Host-side constant prep from the sketch matrices (per-core slices)."""
    Phi = np.zeros((4, C, KPAD), np.float32)
    for i, S in enumerate((S1, S2)):
        F = np.fft.fft(S.astype(np.float64), axis=1)[:, :KH]
        Phi[2 * i, :, :KH] = F.real.astype(np.float32)
        Phi[2 * i + 1, :, :KH] = F.imag.astype(np.float32)

    k = np.arange(KPAD, dtype=np.float64)
    wk = np.where((k == 0) | (k == D // 2), 1.0, 2.0) / D
    wk[KH:] = 0.0
    ang = 2.0 * np.pi * np.outer(k, np.arange(D, dtype=np.float64)) / D
    Cst = np.stack([wk[:, None] * np.cos(ang), -wk[:, None] * np.sin(ang)])
    Cst = Cst.astype(ml_dtypes.bfloat16)  # [2, KPAD, D]

    # phi_packed[j]: [128, (cc, m, kk)] = Phi[m, cc*128+p, 512j+kk]
    arr = Phi.reshape(4, NCC, 128, NCORES, KSL)  # [m, cc, p, j, kk]
    phis, ccs = [], []
    for j in range(NCORES):
        a = arr[:, :, :, j]                      # [m, cc, p, kk]
        a = np.ascontiguousarray(np.transpose(a, (1, 0, 2, 3)))  # [cc, m, p, kk]
        phis.append(np.ascontiguousarray(
            a.transpose(2, 0, 1, 3).reshape(128, NCC * 4 * KSL)))
        c = Cst.reshape(2, NCORES, NKT, 128, D)[:, j]  # [2, kt, 128, D]
        ccs.append(np.ascontiguousarray(c))

    # indicators: [128, (pattern, pt, 2B)] with Re cols 0:16, Im cols 16:32
    # pattern 0 = [+1|0] (RR), 1 = [-1|0] (II), 2 = [0|+1] (RI, IR)
    ind = np.zeros((128, 3 * NPT * 2 * B), np.float32)
    for pt in range(NPT):
        for r in range(min(128, BP - pt * 128)):
            b = (pt * 128 + r) // P
            ind[r, (0 * NPT + pt) * 2 * B + b] = 1.0
            ind[r, (1 * NPT + pt) * 2 * B + b] = -1.0
            ind[r, (2 * NPT + pt) * 2 * B + B + b] = 1.0
    return phis, ccs, ind


_CACHE = {}


def kernel(x, S1, S2):
    x = np.asarray(x)
    if "k" not in _CACHE:
        phis, ccs, ind = make_constants(np.asarray(S1), np.asarray(S2))
        _CACHE["k"] = (build_nc(), phis, ccs, ind)
    nc, phis, ccs, ind = _CACHE["k"]

    xr = np.ascontiguousarray(x.reshape(B, C, P).astype(np.float32))
    in_maps = [
        {"x": xr, "phi": phis[j], "ind": ind, "cc": ccs[j]}
        for j in range(NCORES)
    ]
    res = run_bass_kernel_spmd(nc, in_maps, list(range(NCORES)))
    out = np.zeros((B, D), np.float32)
    for r in res.results:
        out += r["out"]
    return out.astype(x.dtype)



# revision 15
# speedup vs baseline: 1.7410x; 1.7410x over previous
"""Compact Bilinear Pooling (count-sketch + circular conv + spatial sum-pool)
as a Trainium2 Bass/Tile kernel, SPMD over 8 NeuronCores.

Math: with sk_i = flat @ S_i (flat: [B*P, C]), the reference computes
    out[b] = sum_{p in sample b} ifft( fft(sk1_p) * fft(sk2_p) ).real
Fold the (constant) sketch matrices into the DFT:  Phi_i = fft(S_i, axis=1),
so fft(sk_i,p) = x_p^T Phi_i.  Only the half spectrum k = 0..D/2 is needed:
    Shat[b,k]  = sum_p (x_p^T Phi1[:,k]) * (x_p^T Phi2[:,k])
    out[b,d]   = sum_k wk*( Re Shat * cos(2 pi k d/D) - Im Shat * sin(...) )

Per-core pipeline (sharded by frequency: 512 of 4096 padded freqs), all bf16
matmul operands (fp32 PSUM accumulate):
  1. G tiles [128 pos, 512 freq] via bf16 matmuls; PSUM->SBUF bf16 casts
     split over ACT (m0,m1) and DVE (m2,m3).
  2. complex cross-products on DVE (bf16, 2x mode); per-sample position
     reduction via transposed +-1 indicator matmuls with free dim 16
     (out [128 freq, 16 samp]) accumulating into one PSUM tile
     s_acc [128 k, (kt,Re|Im,b)] -> Shat already in [freq, sample] layout.
  3. inverse DFT via Cooley-Tukey split d = d1*1000 + d2 (D1=8):
     alpha-rotation (angle 2 pi k d1/8, depends only on k mod 8) applied on
     DVE -> T1/T2 [128 k, (d1,b)]; then bf16 matmuls against wk*cos/sin
     beta tables [128 k, 1000 d2] -> out partial [(d1 b), d2] per core;
     host sums the 8 partials.
"""

import numpy as np
import ml_dtypes

import concourse.bacc as bacc
import concourse.mybir as mybir
import concourse.tile as tile
from concourse.bass_utils import run_bass_kernel_spmd

# problem dims (hardcoded per spec)
B, C, H, W, D = 16, 512, 14, 14, 8000
P = H * W            # 196 positions per sample
BP = B * P           # 3136
KH = D // 2 + 1      # 4001 half-spectrum frequencies
KPAD = 4096          # padded to 8*512
NCORES = 8
KSL = KPAD // NCORES  # 512 freqs per core
NCC = C // 128        # 4 contraction chunks
NPT = (BP + 127) // 128  # 25 position tiles (zero-padded to 3200)
NKT = KSL // 128      # 4 k-chunks of the core's freq slice
D1 = 8                # inverse DFT radix-split: d = d1*D2 + d2
D2 = D // D1          # 1000
DH = D2 // 2          # 500 (two PSUM halves for the inverse)

# x DMA groups (pt ranges) for pipelined startup
XGRP = [(0, 1), (1, 2), (3, 4), (7, 8), (15, 10)]  # (start_pt, n_pts)

F32 = mybir.dt.float32
BF16 = mybir.dt.bfloat16


def build_nc():
    nc = bacc.Bacc("TRN2", target_bir_lowering=False, debug=False)
    x_d = nc.dram_tensor("x", [NPT, 128, NCC * 128], BF16, kind="ExternalInput")
    phi_d = nc.dram_tensor("phi", [128, NCC * 4 * KSL], BF16, kind="ExternalInput")
    ind_d = nc.dram_tensor("ind", [128, 2 * NPT * B], BF16, kind="ExternalInput")
    atab_d = nc.dram_tensor("atab", [128, 2 * D1 * B], BF16, kind="ExternalInput")
    wb_d = nc.dram_tensor("wb", [128, 2 * NKT * D2], BF16, kind="ExternalInput")
    # device layout [half, (d1 b), d2half]; host transposes to [B, D]
    out_d = nc.dram_tensor("out", [2, 128, DH], F32, kind="ExternalOutput")

    with tile.TileContext(nc) as tc:
        with (
            tc.tile_pool(name="const", bufs=1) as c_pool,
            tc.tile_pool(name="xin", bufs=1) as x_pool,
            tc.tile_pool(name="sbc", bufs=8) as sb_pool,
            tc.tile_pool(name="prd", bufs=8) as pr_pool,
            tc.tile_pool(name="tail", bufs=1) as t_pool,
            tc.tile_pool(name="mm", bufs=5, space="PSUM") as mm_psum,
            tc.tile_pool(name="sac", bufs=1, space="PSUM") as s_psum,
            tc.tile_pool(name="inv", bufs=2, space="PSUM") as inv_psum,
        ):
            # ---- constant / input loads (SP queue order == DMA order)
            indt = c_pool.tile([128, 2 * NPT * B], BF16, tag="ind")
            phit = c_pool.tile([128, NCC * 4 * KSL], BF16, tag="phi")
            atabt = c_pool.tile([128, 2 * D1 * B], BF16, tag="atab")
            wbt = c_pool.tile([128, 2 * NKT * D2], BF16, tag="wb")
            xg = []
            for gi, (st, n) in enumerate(XGRP):
                t = x_pool.tile([128, n * NCC * 128], BF16, tag=f"x{gi}",
                                name=f"xg{gi}")
                xg.append(t)
            src = x_d.ap().rearrange("t c f -> c t f")  # [128, pt, 512]

            def load_xg(gi):
                st, n = XGRP[gi]
                nc.sync.dma_start(
                    xg[gi][:].rearrange("c (t f) -> c t f", t=n),
                    src[:, st:st + n])

            nc.sync.dma_start(indt[:], ind_d.ap())
            load_xg(0)
            for cc in range(NCC):
                sl = slice(cc * 4 * KSL, (cc + 1) * 4 * KSL)
                nc.sync.dma_start(phit[:, sl], phi_d.ap()[:, sl])
            for gi in range(1, len(XGRP)):
                load_xg(gi)
            nc.sync.dma_start(atabt[:], atab_d.ap())
            nc.sync.dma_start(wbt[:], wb_d.ap())

            def x_slice(pt, cc):
                for gi, (st, n) in enumerate(XGRP):
                    if st <= pt < st + n:
                        off = ((pt - st) * NCC + cc) * 128
                        return xg[gi][:, off:off + 128]
                raise AssertionError(pt)

            # ---- Shat accumulator: [128 k0, (kt, Re 16 | Im 16)]
            s_acc = s_psum.tile([128, NKT * 2 * B], F32, tag="sacc")

            def emit_ind(pt, prods):
                # prods = (RR, II, RI, IR); patterns: P+ = ind cols 0, P- = 1
                for i, (pr, pat, imoff) in enumerate((
                    (prods[0], 0, 0), (prods[1], 1, 0),
                    (prods[2], 0, B), (prods[3], 0, B),
                )):
                    roff = (pat * NPT + pt) * B
                    for kt in range(NKT):
                        # PSUM start=True marks the whole 2KB bank pending-zero
                        # (lazily applied on first touch), so exactly ONE start
                        # for the tile; each region's first write then zeroes.
                        nc.tensor.matmul(
                            s_acc[:, kt * 2 * B + imoff: kt * 2 * B + imoff + B],
                            lhsT=pr[:, kt * 128:(kt + 1) * 128],
                            rhs=indt[:, roff:roff + B],
                            start=(pt == 0 and i == 0 and kt == 0),
                            stop=(pt == NPT - 1 and i == 3 and kt == NKT - 1),
                            skip_group_check=True,
                        )

            # ---- main stage (software-pipelined: ind matmuls lag one pt)
            pending = None
            for pt in range(NPT):
                g = []
                for m in range(4):  # 0:g1re 1:g1im 2:g2re 3:g2im
                    gm = mm_psum.tile([128, KSL], F32, tag="mm")
                    for cc in range(NCC):
                        nc.tensor.matmul(
                            gm[:],
                            lhsT=x_slice(pt, cc),
                            rhs=phit[:, (cc * 4 + m) * KSL:(cc * 4 + m + 1) * KSL],
                            start=(cc == 0),
                            stop=(cc == NCC - 1),
                        )
                    g.append(gm)
                if pending is not None:
                    emit_ind(pt - 1, pending)
                # PSUM -> SBUF bf16 casts: m0,m1 on ACT; m2,m3 on DVE
                sb = [sb_pool.tile([128, KSL], BF16, tag="sb", name=f"sb{pt}_{m}")
                      for m in range(4)]
                nc.scalar.copy(sb[0][:], g[0][:])
                nc.scalar.copy(sb[1][:], g[1][:])
                nc.vector.tensor_copy(sb[2][:], g[2][:])
                # products on DVE (bf16 2x); RR/IR only need sb2 -> before cast3
                prods = [pr_pool.tile([128, KSL], BF16, tag="pr", name=f"pr{pt}_{m}")
                         for m in range(4)]
                nc.vector.tensor_mul(prods[0][:], sb[0][:], sb[2][:])  # RR
                nc.vector.tensor_mul(prods[3][:], sb[1][:], sb[2][:])  # IR
                nc.vector.tensor_copy(sb[3][:], g[3][:])
                nc.vector.tensor_mul(prods[1][:], sb[1][:], sb[3][:])  # II
                nc.vector.tensor_mul(prods[2][:], sb[0][:], sb[3][:])  # RI
                pending = prods
            emit_ind(NPT - 1, pending)

            # ---- tail: alpha rotation (DVE) + inverse beta matmuls
            cosA = atabt[:, 0:D1 * B].rearrange("k (a b) -> k a b", a=D1)
            sinAm = atabt[:, D1 * B:].rearrange("k (a b) -> k a b", a=D1)
            T1s, T2s = [], []
            for kt in range(NKT):
                sT = t_pool.tile([128, 2 * B], BF16, tag=f"sT{kt}")
                nc.scalar.copy(sT[:], s_acc[:, kt * 2 * B:(kt + 1) * 2 * B])
                sre = sT[:, 0:B].rearrange("k (o b) -> k o b", o=1) \
                    .broadcast_to([128, D1, B])
                sim = sT[:, B:2 * B].rearrange("k (o b) -> k o b", o=1) \
                    .broadcast_to([128, D1, B])
                m1 = t_pool.tile([128, D1 * B], BF16, tag="m1", bufs=1)
                m2 = t_pool.tile([128, D1 * B], BF16, tag="m2", bufs=1)
                T1 = t_pool.tile([128, D1 * B], BF16, tag=f"T1k{kt}")
                T2 = t_pool.tile([128, D1 * B], BF16, tag=f"T2k{kt}")
                m1v = m1[:].rearrange("k (a b) -> k a b", a=D1)
                m2v = m2[:].rearrange("k (a b) -> k a b", a=D1)
                # T1 = cosA*Sre + (-sinA)*Sim ; T2 = (-sinA)*Sre - cosA*Sim
                nc.vector.tensor_mul(m1v, cosA, sre)
                nc.vector.tensor_mul(m2v, sinAm, sim)
                nc.vector.tensor_add(T1[:].rearrange("k (a b) -> k a b", a=D1), m1v, m2v)
                nc.vector.tensor_mul(m1v, sinAm, sre)
                nc.vector.tensor_mul(m2v, cosA, sim)
                nc.vector.tensor_sub(T2[:].rearrange("k (a b) -> k a b", a=D1), m1v, m2v)
                T1s.append(T1)
                T2s.append(T2)

            outv = out_d.ap()
            for half in range(2):
                pinv = inv_psum.tile([128, DH], F32, tag="inv")
                idx = 0
                for kt in range(NKT):
                    for t, Ts in ((0, T1s), (1, T2s)):
                        nc.tensor.matmul(
                            pinv[:],
                            lhsT=Ts[kt][:],
                            rhs=wbt[:, (t * NKT + kt) * D2 + half * DH:
                                    (t * NKT + kt) * D2 + half * DH + DH],
                            start=(idx == 0),
                            stop=(idx == 7),
                        )
                        idx += 1
                stage = t_pool.tile([128, DH], F32, tag="stage", bufs=1,
                                    name=f"stage{half}")
                nc.scalar.copy(stage[:], pinv[:])
                nc.sync.dma_start(outv[half], stage[:])

    nc.compile()
    return nc


def make_constants(S1, S2):
    """Host-side constant prep from the sketch matrices (per-core slices)."""
    Phi = np.zeros((4, C, KPAD), np.float32)
    for i, S in enumerate((S1, S2)):
        F = np.fft.fft(S.astype(np.float64), axis=1)[:, :KH]
        Phi[2 * i, :, :KH] = F.real.astype(np.float32)
        Phi[2 * i + 1, :, :KH] = F.imag.astype(np.float32)

    # phi[j]: [c0, (cc, m, kk)]
    arr = Phi.reshape(4, NCC, 128, NCORES, KSL)  # [m, cc, c0, j, kk]
    phis = []
    for j in range(NCORES):
        a = np.transpose(arr[:, :, :, j], (2, 1, 0, 3))  # [c0, cc, m, kk]
        phis.append(np.ascontiguousarray(
            a.reshape(128, NCC * 4 * KSL).astype(ml_dtypes.bfloat16)))

    # indicators [128, (pat, pt, b)]: pat0 = +1 at col b, pat1 = -1 at col b
    ind = np.zeros((128, 2 * NPT * B), np.float32)
    for pt in range(NPT):
        for r in range(min(128, BP - pt * 128)):
            b = (pt * 128 + r) // P
            ind[r, (0 * NPT + pt) * B + b] = 1.0
            ind[r, (1 * NPT + pt) * B + b] = -1.0
    ind = ind.astype(ml_dtypes.bfloat16)

    # alpha tables [128 k0, (fn, d1, b)] (b-replicated; depends on k0 mod 8)
    k0 = np.arange(128)
    d1 = np.arange(D1)
    ang = 2.0 * np.pi * np.outer(k0 % D1, d1) / D1  # [128, 8]
    atab = np.zeros((128, 2 * D1 * B), np.float64)
    atab[:, 0:D1 * B] = np.repeat(np.cos(ang), B, axis=1)
    atab[:, D1 * B:] = np.repeat(-np.sin(ang), B, axis=1)
    atab = atab.astype(ml_dtypes.bfloat16)

    # beta tables per core: [128 k0, (t, kt, d2)] = wk * {cos,sin}(2 pi k d2/D)
    wbs = []
    d2 = np.arange(D2, dtype=np.float64)
    for j in range(NCORES):
        wb = np.zeros((128, 2 * NKT * D2), np.float64)
        for kt in range(NKT):
            k = j * KSL + kt * 128 + k0  # [128]
            wk = np.where((k == 0) | (k == D // 2), 1.0, 2.0) / D
            wk[k >= KH] = 0.0
            bang = 2.0 * np.pi * np.outer(k, d2) / D  # [128, 1000]
            wb[:, (0 * NKT + kt) * D2:(0 * NKT + kt + 1) * D2] = \
                wk[:, None] * np.cos(bang)
            wb[:, (1 * NKT + kt) * D2:(1 * NKT + kt + 1) * D2] = \
                wk[:, None] * np.sin(bang)
        wbs.append(np.ascontiguousarray(wb.astype(ml_dtypes.bfloat16)))
    return phis, ind, atab, wbs


def pack_x(x):
    """x [B,C,H,W] f32 -> [NPT, 128 c0, NCC*128] bf16, positions zero-padded."""
    xt = x.reshape(B, C, P).transpose(1, 0, 2).reshape(C, BP)
    xp = np.zeros((C, NPT * 128), np.float32)
    xp[:, :BP] = xt
    x2 = xp.reshape(NCC, 128, NPT, 128).transpose(2, 1, 0, 3)  # [pt,c0,cc,p]
    return np.ascontiguousarray(
        x2.reshape(NPT, 128, NCC * 128).astype(ml_dtypes.bfloat16))


_CACHE = {}


def kernel(x, S1, S2):
    x = np.asarray(x)
    if "k" not in _CACHE:
        phis, ind, atab, wbs = make_constants(np.asarray(S1), np.asarray(S2))
        _CACHE["k"] = (build_nc(), phis, ind, atab, wbs)
    nc, phis, ind, atab, wbs = _CACHE["k"]

    x2 = pack_x(x.astype(np.float32))
    in_maps = [
        {"x": x2, "phi": phis[j], "ind": ind, "atab": atab, "wb": wbs[j]}
        for j in range(NCORES)
    ]
    res = run_bass_kernel_spmd(nc, in_maps, list(range(NCORES)))
    acc = np.zeros((2, 128, DH), np.float32)
    for r in res.results:
        acc += r["out"]
    # [h, (d1 b), e] -> [b, (d1 h e)]
    out = np.transpose(acc.reshape(2, D1, B, DH), (2, 1, 0, 3)).reshape(B, D)
    return np.ascontiguousarray(out).astype(x.dtype)


# revision 37
# speedup vs baseline: 1.7422x; 1.0007x over previous
"""Compact Bilinear Pooling (count-sketch + circular conv + spatial sum-pool)
as a Trainium2 Bass/Tile kernel, SPMD over 8 NeuronCores.

Math: with sk_i = flat @ S_i (flat: [B*P, C]), the reference computes
    out[b] = sum_{p in sample b} ifft( fft(sk1_p) * fft(sk2_p) ).real
Fold the (constant) sketch matrices into the DFT:  Phi_i = fft(S_i, axis=1),
so fft(sk_i,p) = x_p^T Phi_i.  Only the half spectrum k = 0..D/2 is needed:
    Shat[b,k]  = sum_p (x_p^T Phi1[:,k]) * (x_p^T Phi2[:,k])
    out[b,d]   = sum_k wk*( Re Shat * cos(2 pi k d/D) - Im Shat * sin(...) )

Per-core pipeline (sharded by frequency: 512 of 4096 padded freqs), all bf16
matmul operands (fp32 PSUM accumulate):
  1. G tiles [128 pos, 512 freq] via bf16 matmuls; PSUM->SBUF bf16 casts
     split over ACT (m0,m1) and DVE (m2,m3).
  2. complex cross-products on DVE (bf16, 2x mode); per-sample position
     reduction via transposed +-1 indicator matmuls with free dim 16
     (out [128 freq, 16 samp]) accumulating into one PSUM tile
     s_acc [128 k, (kt,Re|Im,b)] -> Shat already in [freq, sample] layout.
  3. inverse DFT via Cooley-Tukey split d = d1*1000 + d2 (D1=8):
     alpha-rotation (angle 2 pi k d1/8, depends only on k mod 8) applied on
     DVE -> T1/T2 [128 k, (d1,b)]; then bf16 matmuls against wk*cos/sin
     beta tables [128 k, 1000 d2] -> out partial [(d1 b), d2] per core;
     host sums the 8 partials.
"""

import numpy as np
import ml_dtypes

import concourse.bacc as bacc
import concourse.mybir as mybir
import concourse.tile as tile
from concourse.bass_utils import run_bass_kernel_spmd

# problem dims (hardcoded per spec)
B, C, H, W, D = 16, 512, 14, 14, 8000
P = H * W            # 196 positions per sample
BP = B * P           # 3136
KH = D // 2 + 1      # 4001 half-spectrum frequencies
KPAD = 4096          # padded to 8*512
NCORES = 8
KSL = KPAD // NCORES  # 512 freqs per core
NCC = C // 128        # 4 contraction chunks
NPT = (BP + 127) // 128  # 25 position tiles (zero-padded to 3200)
NKT = KSL // 128      # 4 k-chunks of the core's freq slice
D1 = 8                # inverse DFT radix-split: d = d1*D2 + d2
D2 = D // D1          # 1000
DH = D2 // 2          # 500 (two PSUM halves for the inverse)

# x DMA groups (pt ranges) for pipelined startup
XGRP = [(0, 1), (1, 2), (3, 4), (7, 8), (15, 10)]  # (start_pt, n_pts)
NSPLIT = 25           # <25 would split Shat into A/B halves (measured slower)
INTERLEAVE_A = False  # emit A-half inverse inside the main loop (slower)
PHI_MM = True         # phi DMA sliced m-major: first chunk unblocks m0 group
BRIDGE = False        # PE p-state bridge matmuls before the tail (no effect)

F32 = mybir.dt.float32
BF16 = mybir.dt.bfloat16


def build_nc():
    nc = bacc.Bacc("TRN2", target_bir_lowering=False, debug=False)
    x_d = nc.dram_tensor("x", [NPT, 128, NCC * 128], BF16, kind="ExternalInput")
    phi_d = nc.dram_tensor("phi", [128, NCC * 4 * KSL], BF16, kind="ExternalInput")
    ind_d = nc.dram_tensor("ind", [128, 2 * NPT * B], BF16, kind="ExternalInput")
    atab_d = nc.dram_tensor("atab", [128, 2 * D1 * B], BF16, kind="ExternalInput")
    wb_d = nc.dram_tensor("wb", [128, 2 * NKT * D2], BF16, kind="ExternalInput")
    # device layout [half, (d1 b), d2half]; host transposes to [B, D]
    out_d = nc.dram_tensor("out", [2, 128, DH], F32, kind="ExternalOutput")

    with tile.TileContext(nc) as tc:
        with (
            tc.tile_pool(name="const", bufs=1) as c_pool,
            tc.tile_pool(name="xin", bufs=1) as x_pool,
            tc.tile_pool(name="sbc", bufs=8) as sb_pool,
            tc.tile_pool(name="prd", bufs=8) as pr_pool,
            tc.tile_pool(name="tail", bufs=1) as t_pool,
            tc.tile_pool(name="mm", bufs=5, space="PSUM") as mm_psum,
            tc.tile_pool(name="sac", bufs=1, space="PSUM") as s_psum,
            tc.tile_pool(name="inv", bufs=1, space="PSUM") as inv_psum,
        ):
            # ---- constant / input loads (SP queue order == DMA order)
            indt = c_pool.tile([128, 2 * NPT * B], BF16, tag="ind")
            phit = c_pool.tile([128, NCC * 4 * KSL], BF16, tag="phi")
            atabt = c_pool.tile([128, 2 * D1 * B], BF16, tag="atab")
            wbt = c_pool.tile([128, 2 * NKT * D2], BF16, tag="wb")
            xg = []
            for gi, (st, n) in enumerate(XGRP):
                t = x_pool.tile([128, n * NCC * 128], BF16, tag=f"x{gi}",
                                name=f"xg{gi}")
                xg.append(t)
            src = x_d.ap().rearrange("t c f -> c t f")  # [128, pt, 512]

            def load_xg(gi):
                st, n = XGRP[gi]
                nc.sync.dma_start(
                    xg[gi][:].rearrange("c (t f) -> c t f", t=n),
                    src[:, st:st + n])

            # phi sliced m-major: the first chunk completes the whole m0
            # accumulation group so pt0's matmuls start after ~2MB/4 of DMA
            phiv_t = phit[:].rearrange("c (cc m k) -> c cc m k", cc=NCC, m=4)
            phiv_d = phi_d.ap().rearrange("c (cc m k) -> c cc m k", cc=NCC, m=4)
            if PHI_MM:
                load_xg(0)
                nc.sync.dma_start(phiv_t[:, :, 0], phiv_d[:, :, 0])
                nc.sync.dma_start(indt[:], ind_d.ap())
                for m in range(1, 4):
                    nc.sync.dma_start(phiv_t[:, :, m], phiv_d[:, :, m])
            else:
                nc.sync.dma_start(indt[:], ind_d.ap())
                load_xg(0)
                for cc in range(NCC):
                    sl = slice(cc * 4 * KSL, (cc + 1) * 4 * KSL)
                    nc.sync.dma_start(phit[:, sl], phi_d.ap()[:, sl])
            for gi in range(1, len(XGRP)):
                load_xg(gi)
            nc.sync.dma_start(atabt[:], atab_d.ap())
            nc.sync.dma_start(wbt[:], wb_d.ap())

            def x_slice(pt, cc):
                for gi, (st, n) in enumerate(XGRP):
                    if st <= pt < st + n:
                        off = ((pt - st) * NCC + cc) * 128
                        return xg[gi][:, off:off + 128]
                raise AssertionError(pt)

            # ---- Shat accumulator: A half cols 0:128, B half cols 128:256
            # (one PSUM bank; a single start marks the whole bank pending-zero
            # so every region's first write zero-fills — no second start)
            s_acc2 = s_psum.tile([128, 2 * NKT * 2 * B], F32, tag="sacc")
            s_accA = s_acc2[:, 0:NKT * 2 * B]
            s_accB = s_acc2[:, NKT * 2 * B:]
            pinv = [inv_psum.tile([128, DH], F32, tag=f"inv{h}", name=f"pinv{h}")
                    for h in range(2)]

            def emit_ind(pt, prods):
                # prods = (RR, II, RI, IR); patterns: P+ = ind cols 0, P- = 1
                s_acc = s_accA if pt < NSPLIT else s_accB
                first = pt == 0
                last = pt == NPT - 1
                for i, (pr, pat, imoff) in enumerate((
                    (prods[0], 0, 0), (prods[1], 1, 0),
                    (prods[2], 0, B), (prods[3], 0, B),
                )):
                    roff = (pat * NPT + pt) * B
                    for kt in range(NKT):
                        # PSUM start=True marks the whole 2KB bank pending-zero
                        # (lazily applied on first touch), so exactly ONE start
                        # for the tile; each region's first write then zeroes.
                        nc.tensor.matmul(
                            s_acc[:, kt * 2 * B + imoff: kt * 2 * B + imoff + B],
                            lhsT=pr[:, kt * 128:(kt + 1) * 128],
                            rhs=indt[:, roff:roff + B],
                            start=(first and i == 0 and kt == 0),
                            stop=(last and i == 3 and kt == NKT - 1),
                            skip_group_check=True,
                        )

            cosA = atabt[:, 0:D1 * B].rearrange("k (a b) -> k a b", a=D1)
            sinAm = atabt[:, D1 * B:].rearrange("k (a b) -> k a b", a=D1)

            def emit_inv_kt(ph, kt, Ts):
                """4 inverse matmuls for one kt (lhsT = previously built T1/T2)."""
                T1, T2 = Ts
                for half in range(2):
                    for t, T in ((0, T1), (1, T2)):
                        nc.tensor.matmul(
                            pinv[half][:],
                            lhsT=T[:],
                            rhs=wbt[:, (t * NKT + kt) * D2 + half * DH:
                                    (t * NKT + kt) * D2 + half * DH + DH],
                            start=(ph == "A" and kt == 0 and t == 0),
                            stop=(ph == ("B" if NSPLIT < NPT else "A")
                                  and kt == NKT - 1 and t == 1),
                            skip_group_check=True,
                        )

            def emit_alpha_kt(ph, kt, s_acc):
                """sT cast + alpha rotation for one kt; returns (T1, T2)."""
                sT = t_pool.tile([128, 2 * B], BF16, tag=f"sT{ph}{kt}",
                                 name=f"sT{ph}{kt}")
                nc.scalar.copy(sT[:], s_acc[:, kt * 2 * B:(kt + 1) * 2 * B])
                sre = sT[:, 0:B].rearrange("k (o b) -> k o b", o=1) \
                    .broadcast_to([128, D1, B])
                sim = sT[:, B:2 * B].rearrange("k (o b) -> k o b", o=1) \
                    .broadcast_to([128, D1, B])
                m1 = t_pool.tile([128, D1 * B], BF16, tag="m1", bufs=1, name="m1")
                m2 = t_pool.tile([128, D1 * B], BF16, tag="m2", bufs=1, name="m2")
                T1 = t_pool.tile([128, D1 * B], BF16, tag=f"T1{ph}{kt}",
                                 name=f"T1{ph}{kt}")
                T2 = t_pool.tile([128, D1 * B], BF16, tag=f"T2{ph}{kt}",
                                 name=f"T2{ph}{kt}")
                m1v = m1[:].rearrange("k (a b) -> k a b", a=D1)
                m2v = m2[:].rearrange("k (a b) -> k a b", a=D1)
                # T1 = cosA*Sre + (-sinA)*Sim ; T2 = (-sinA)*Sre - cosA*Sim
                nc.vector.tensor_mul(m1v, cosA, sre)
                nc.vector.tensor_mul(m2v, sinAm, sim)
                nc.vector.tensor_add(T1[:].rearrange("k (a b) -> k a b", a=D1),
                                     m1v, m2v)
                nc.vector.tensor_mul(m1v, sinAm, sre)
                nc.vector.tensor_mul(m2v, cosA, sim)
                nc.vector.tensor_sub(T2[:].rearrange("k (a b) -> k a b", a=D1),
                                     m1v, m2v)
                return (T1, T2)

            # ---- main stage (software-pipelined: ind matmuls lag one pt;
            #      A-half inverse interleaved one kt per iteration)
            pending = None
            TsA = []
            for pt in range(NPT):
                g = []
                for m in range(4):  # 0:g1re 1:g1im 2:g2re 3:g2im
                    gm = mm_psum.tile([128, KSL], F32, tag="mm")
                    for cc in range(NCC):
                        nc.tensor.matmul(
                            gm[:],
                            lhsT=x_slice(pt, cc),
                            rhs=phit[:, (cc * 4 + m) * KSL:(cc * 4 + m + 1) * KSL],
                            start=(cc == 0),
                            stop=(cc == NCC - 1),
                        )
                    g.append(gm)
                if pending is not None:
                    emit_ind(pt - 1, pending)
                # PSUM -> SBUF bf16 casts: m0,m1 on ACT; m2,m3 on DVE
                sb = [sb_pool.tile([128, KSL], BF16, tag="sb", name=f"sb{pt}_{m}")
                      for m in range(4)]
                nc.scalar.copy(sb[0][:], g[0][:])
                nc.scalar.copy(sb[1][:], g[1][:])
                nc.vector.tensor_copy(sb[2][:], g[2][:])
                # products on DVE (bf16 2x); RR/IR only need sb2 -> before cast3
                prods = [pr_pool.tile([128, KSL], BF16, tag="pr", name=f"pr{pt}_{m}")
                         for m in range(4)]
                nc.vector.tensor_mul(prods[0][:], sb[0][:], sb[2][:])  # RR
                nc.vector.tensor_mul(prods[3][:], sb[1][:], sb[2][:])  # IR
                nc.vector.tensor_copy(sb[3][:], g[3][:])
                nc.vector.tensor_mul(prods[1][:], sb[1][:], sb[3][:])  # II
                nc.vector.tensor_mul(prods[2][:], sb[0][:], sb[3][:])  # RI
                pending = prods
                if INTERLEAVE_A:
                    # alpha for kt at iteration NSPLIT+1+kt; its inverse
                    # matmuls one iteration later so the PE never waits on DVE
                    if NSPLIT + 1 <= pt <= NSPLIT + NKT:
                        TsA.append(emit_alpha_kt("A", pt - (NSPLIT + 1), s_accA))
                    if NSPLIT + 2 <= pt <= NSPLIT + 1 + NKT:
                        emit_inv_kt("A", pt - (NSPLIT + 2), TsA[pt - (NSPLIT + 2)])
            emit_ind(NPT - 1, pending)
            if BRIDGE:
                # keep the PE p-state warm across the gap while the first
                # alpha rotation (DVE) runs (results unused)
                scratch = mm_psum.tile([128, KSL], F32, tag="mm", name="scratch")
                for i in range(4):
                    nc.tensor.matmul(
                        scratch[:], lhsT=phit[:, 0:128], rhs=phit[:, 0:KSL],
                        start=(i == 0), stop=(i == 3), skip_group_check=True)
            if not INTERLEAVE_A:
                for kt in range(NKT):
                    emit_inv_kt("A", kt, emit_alpha_kt("A", kt, s_accA))

            # bridge matmuls: keep the PE p-state warm across the short gap
            # before the B-half inverse (results unused)
            if NSPLIT < NPT:
                TsB = [emit_alpha_kt("B", kt, s_accB) for kt in range(NKT)]
                for kt in range(NKT):
                    emit_inv_kt("B", kt, TsB[kt])

            outv = out_d.ap()
            for half in range(2):
                stage = t_pool.tile([128, DH], F32, tag="stage", bufs=1,
                                    name=f"stage{half}")
                nc.scalar.copy(stage[:], pinv[half][:])
                nc.sync.dma_start(outv[half], stage[:])

    nc.compile()
    return nc


def make_constants(S1, S2):
    """Host-side constant prep from the sketch matrices (per-core slices)."""
    Phi = np.zeros((4, C, KPAD), np.float32)
    for i, S in enumerate((S1, S2)):
        F = np.fft.fft(S.astype(np.float64), axis=1)[:, :KH]
        Phi[2 * i, :, :KH] = F.real.astype(np.float32)
        Phi[2 * i + 1, :, :KH] = F.imag.astype(np.float32)

    # phi[j]: [c0, (cc, m, kk)]
    arr = Phi.reshape(4, NCC, 128, NCORES, KSL)  # [m, cc, c0, j, kk]
    phis = []
    for j in range(NCORES):
        a = np.transpose(arr[:, :, :, j], (2, 1, 0, 3))  # [c0, cc, m, kk]
        phis.append(np.ascontiguousarray(
            a.reshape(128, NCC * 4 * KSL).astype(ml_dtypes.bfloat16)))

    # indicators [128, (pat, pt, b)]: pat0 = +1 at col b, pat1 = -1 at col b
    ind = np.zeros((128, 2 * NPT * B), np.float32)
    for pt in range(NPT):
        for r in range(min(128, BP - pt * 128)):
            b = (pt * 128 + r) // P
            ind[r, (0 * NPT + pt) * B + b] = 1.0
            ind[r, (1 * NPT + pt) * B + b] = -1.0
    ind = ind.astype(ml_dtypes.bfloat16)

    # alpha tables [128 k0, (fn, d1, b)] (b-replicated; depends on k0 mod 8)
    k0 = np.arange(128)
    d1 = np.arange(D1)
    ang = 2.0 * np.pi * np.outer(k0 % D1, d1) / D1  # [128, 8]
    atab = np.zeros((128, 2 * D1 * B), np.float64)
    atab[:, 0:D1 * B] = np.repeat(np.cos(ang), B, axis=1)
    atab[:, D1 * B:] = np.repeat(-np.sin(ang), B, axis=1)
    atab = atab.astype(ml_dtypes.bfloat16)

    # beta tables per core: [128 k0, (t, kt, d2)] = wk * {cos,sin}(2 pi k d2/D)
    wbs = []
    d2 = np.arange(D2, dtype=np.float64)
    for j in range(NCORES):
        wb = np.zeros((128, 2 * NKT * D2), np.float64)
        for kt in range(NKT):
            k = j * KSL + kt * 128 + k0  # [128]
            wk = np.where((k == 0) | (k == D // 2), 1.0, 2.0) / D
            wk[k >= KH] = 0.0
            bang = 2.0 * np.pi * np.outer(k, d2) / D  # [128, 1000]
            wb[:, (0 * NKT + kt) * D2:(0 * NKT + kt + 1) * D2] = \
                wk[:, None] * np.cos(bang)
            wb[:, (1 * NKT + kt) * D2:(1 * NKT + kt + 1) * D2] = \
                wk[:, None] * np.sin(bang)
        wbs.append(np.ascontiguousarray(wb.astype(ml_dtypes.bfloat16)))
    return phis, ind, atab, wbs


def pack_x(x):
    """x [B,C,H,W] f32 -> [NPT, 128 c0, NCC*128] bf16, positions zero-padded."""
    xt = x.reshape(B, C, P).transpose(1, 0, 2).reshape(C, BP)
    xp = np.zeros((C, NPT * 128), np.float32)
    xp[:, :BP] = xt
    x2 = xp.reshape(NCC, 128, NPT, 128).transpose(2, 1, 0, 3)  # [pt,c0,cc,p]
    return np.ascontiguousarray(
        x2.reshape(NPT, 128, NCC * 128).astype(ml_dtypes.bfloat16))


_CACHE = {}


def kernel(x, S1, S2):
    x = np.asarray(x)
    if "k" not in _CACHE:
        phis, ind, atab, wbs = make_constants(np.asarray(S1), np.asarray(S2))
        _CACHE["k"] = (build_nc(), phis, ind, atab, wbs)
    nc, phis, ind, atab, wbs = _CACHE["k"]

    x2 = pack_x(x.astype(np.float32))
    in_maps = [
        {"x": x2, "phi": phis[j], "ind": ind, "atab": atab, "wb": wbs[j]}
        for j in range(NCORES)
    ]
    res = run_bass_kernel_spmd(nc, in_maps, list(range(NCORES)))
    acc = np.zeros((2, 128, DH), np.float32)
    for r in res.results:
        acc += r["out"]
    # [h, (d1 b), e] -> [b, (d1 h e)]
    out = np.transpose(acc.reshape(2, D1, B, DH), (2, 1, 0, 3)).reshape(B, D)
    return np.ascontiguousarray(out).astype(x.dtype)
